# revision 2
# baseline (speedup 1.0000x reference)
"""Distributed Trainium2 kernel for nn_JointModel_46823733461791.

Sharding (per spec hint): the small graph (2137 nodes) + GAT params are
replicated; GAT attention heads are sharded across the 8 cores
(head-parallel, heads are independent); the user batch dim is sharded
for the MLP tail (data parallel). Cross-head dependencies (h1 concat,
h2 mean) are resolved with all-gathers inside the shard_map program.

kernel(**inputs) takes FULL unsharded numpy inputs and returns the FULL
output tuple matching reference.reference().
"""

import numpy as np
import jax
import jax.numpy as jnp
from jax.sharding import Mesh, PartitionSpec as P
from jax.experimental.shard_map import shard_map
from functools import partial

N_P = 2048
N_W = 89
N_ALL = N_P + N_W
B = 16384
E2 = 768
UW = 313
NC = 8

HI = jax.lax.Precision.HIGHEST


def _gat_head(h, adj, w_h, asrc_h, adst_h, bias):
    # h: (n, 256); w_h: (256, 32); asrc/adst: (32,); adj: (n, n) int32
    hp = jnp.einsum("nf,fo->no", h, w_h, precision=HI)          # (n, 32)
    e_src = jnp.einsum("no,o->n", hp, asrc_h, precision=HI)     # (n,)
    e_dst = jnp.einsum("no,o->n", hp, adst_h, precision=HI)     # (n,)
    e = jax.nn.leaky_relu(e_src[:, None] + e_dst[None, :], 0.2)
    e = jnp.where(adj > 0, e, jnp.float32(-1e9))
    attn = jax.nn.softmax(e, axis=-1)                           # (n, n)
    out = jnp.einsum("nm,mo->no", attn, hp, precision=HI) + bias
    return attn, out


def _device_fn(pern_feature, word_feature, pern_adj, word_pern_adj,
               uwa_c, sent_c,
               w1_c, asrc1_c, adst1_c, bias1,
               w2_c, asrc2_c, adst2_c, bias2,
               W_s1, b_s1, W_s2, b_s2, W_f1, b_f1, W_fa, b_fa, W_fb, b_fb):
    # Per-core: one GAT head of each layer + a 1/8 slice of the user batch.
    # Sharded-in args carry a leading axis of size 1 (their shard).
    w1h, a1s, a1d = w1_c[0], asrc1_c[0], adst1_c[0]
    w2h, a2s, a2d = w2_c[0], asrc2_c[0], adst2_c[0]

    # ---- GAT 1 (personality graph), head-parallel ----
    attn1, out1 = _gat_head(pern_feature, pern_adj, w1h, a1s, a1d, bias1)
    # h1 needs all 8 heads: all-gather the small per-head outputs (2048x32).
    out1_all = jax.lax.all_gather(out1, "c", axis=0)            # (8, n, 32)
    h1 = jax.nn.elu(jnp.transpose(out1_all, (1, 0, 2)).reshape(N_P, -1))

    # ---- GAT 2 (word+personality graph), head-parallel ----
    h1_wp = jnp.concatenate([word_feature, h1], axis=0)         # (2137, 256)
    attn2, out2 = _gat_head(h1_wp, word_pern_adj, w2h, a2s, a2d, bias2)
    out2_all = jax.lax.all_gather(out2, "c", axis=0)            # (8, m, 32)
    h2 = out2_all.mean(axis=0)                                  # (2137, 32)
    h2_word = h2[:N_W, :]                                       # (89, 32)

    # ---- user batch (data parallel over the 2048-user shard) ----
    x_user = jnp.einsum("uk,ko->uo", uwa_c[:, :N_W], h2_word, precision=HI)
    x_c = jnp.concatenate([x_user, uwa_c[:, N_W:]], axis=1)     # (bu, 256)
    x = x_c / x_c.sum(axis=1, keepdims=True)
    o1 = jnp.einsum("uf,fj->uj", x, W_s1, precision=HI) + b_s1
    o2 = jnp.einsum("uf,fj->uj", sent_c, W_s2, precision=HI) + b_s2
    o1n = jnp.einsum("uj,jk->uk", o1, W_f1, precision=HI) + b_f1
    o2n = jnp.einsum("uj,jk->uk", o2, W_f1, precision=HI) + b_f1
    xt = jnp.concatenate([o1, o2], axis=1)
    xt = jnp.einsum("uf,fj->uj", jax.nn.relu(
        jnp.einsum("uf,fj->uj", xt, W_fa, precision=HI) + b_fa),
        W_fb, precision=HI) + b_fb
    return (o1n, o2n, xt, attn1[None], attn2[None], x)


_COMPILED = None


def _get_compiled():
    global _COMPILED
    if _COMPILED is not None:
        return _COMPILED
    devs = jax.devices()[:NC]
    mesh = Mesh(np.asarray(devs), ("c",))
    rep = P()
    shd = P("c")
    in_specs = (
        rep, rep, rep, rep,      # pern_feature, word_feature, pern_adj, wp_adj
        shd, shd,                # user_word_adj, sentence_embed (batch dim)
        shd, shd, shd, rep,      # w1, a_src1, a_dst1, bias1   (head dim)
        shd, shd, shd, rep,      # w2, a_src2, a_dst2, bias2
        rep, rep, rep, rep, rep, rep, rep, rep, rep, rep,
    )
    out_specs = (shd, shd, shd,  # o1n, o2n, xt   (batch dim)
                 shd, shd,       # attn1, attn2   (head dim)
                 shd)            # x              (batch dim)
    fn = jax.jit(shard_map(_device_fn, mesh=mesh,
                           in_specs=in_specs, out_specs=out_specs,
                           check_rep=False))
    _COMPILED = fn
    return fn


def kernel(**inputs):
    fn = _get_compiled()
    args = (
        inputs["pern_feature"], inputs["word_feature"],
        inputs["pern_adj"], inputs["word_pern_adj"],
        inputs["user_word_adj"], inputs["sentence_embed"],
        inputs["w1"], inputs["a_src1"], inputs["a_dst1"], inputs["bias1"],
        inputs["w2"], inputs["a_src2"], inputs["a_dst2"], inputs["bias2"],
        inputs["W_s1"], inputs["b_s1"], inputs["W_s2"], inputs["b_s2"],
        inputs["W_f1"], inputs["b_f1"], inputs["W_fa"], inputs["b_fa"],
        inputs["W_fb"], inputs["b_fb"],
    )
    args = tuple(np.asarray(a) for a in args)
    global _last_args
    _last_args = args
    out = fn(*args)
    out = jax.tree.map(np.asarray, out)
    o1n, o2n, xt, attn1, attn2, x = out
    return (o1n, o2n, xt, attn1, attn2, x)


# revision 3
# speedup vs baseline: 87.8708x; 87.8708x over previous
"""Distributed Trainium2 kernel for nn_JointModel_46823733461791.

Sharding (per spec hint): the small graph (2137 nodes) + GAT params are
replicated; GAT attention heads are sharded across the 8 cores
(head-parallel, heads are independent); the user batch dim is sharded
for the MLP tail (data parallel). Cross-head dependencies (h1 concat,
h2 mean) are resolved with all-gathers inside the shard_map program.

kernel(**inputs) takes FULL unsharded numpy inputs and returns the FULL
output tuple matching reference.reference().
"""

import numpy as np
import jax
import jax.numpy as jnp
from jax.sharding import Mesh, PartitionSpec as P
from jax.experimental.shard_map import shard_map
from functools import partial

N_P = 2048
N_W = 89
N_ALL = N_P + N_W
B = 16384
E2 = 768
UW = 313
NC = 8

HI = jax.lax.Precision.HIGHEST


def _gat_head(h, adj, w_h, asrc_h, adst_h, bias):
    # h: (n, 256); w_h: (256, 32); asrc/adst: (32,); adj: (n, n) int32
    hp = jnp.einsum("nf,fo->no", h, w_h, precision=HI)          # (n, 32)
    e_src = jnp.einsum("no,o->n", hp, asrc_h, precision=HI)     # (n,)
    e_dst = jnp.einsum("no,o->n", hp, adst_h, precision=HI)     # (n,)
    e = jax.nn.leaky_relu(e_src[:, None] + e_dst[None, :], 0.2)
    e = jnp.where(adj > 0, e, jnp.float32(-1e9))
    attn = jax.nn.softmax(e, axis=-1)                           # (n, n)
    out = jnp.einsum("nm,mo->no", attn, hp) + bias
    return attn, out


def _device_fn(pern_feature, word_feature, pern_adj, word_pern_adj,
               uwa_c, sent_c,
               w1_c, asrc1_c, adst1_c, bias1,
               w2_c, asrc2_c, adst2_c, bias2,
               W_s1, b_s1, W_s2, b_s2, W_f1, b_f1, W_fa, b_fa, W_fb, b_fb):
    # Per-core: one GAT head of each layer + a 1/8 slice of the user batch.
    # Sharded-in args carry a leading axis of size 1 (their shard).
    w1h, a1s, a1d = w1_c[0], asrc1_c[0], adst1_c[0]
    w2h, a2s, a2d = w2_c[0], asrc2_c[0], adst2_c[0]

    # ---- GAT 1 (personality graph), head-parallel ----
    attn1, out1 = _gat_head(pern_feature, pern_adj, w1h, a1s, a1d, bias1)
    # h1 needs all 8 heads: all-gather the small per-head outputs (2048x32).
    out1_all = jax.lax.all_gather(out1, "c", axis=0)            # (8, n, 32)
    h1 = jax.nn.elu(jnp.transpose(out1_all, (1, 0, 2)).reshape(N_P, -1))

    # ---- GAT 2 (word+personality graph), head-parallel ----
    h1_wp = jnp.concatenate([word_feature, h1], axis=0)         # (2137, 256)
    attn2, out2 = _gat_head(h1_wp, word_pern_adj, w2h, a2s, a2d, bias2)
    out2_all = jax.lax.all_gather(out2, "c", axis=0)            # (8, m, 32)
    h2 = out2_all.mean(axis=0)                                  # (2137, 32)
    h2_word = h2[:N_W, :]                                       # (89, 32)

    # ---- user batch (data parallel over the 2048-user shard) ----
    x_user = jnp.einsum("uk,ko->uo", uwa_c[:, :N_W], h2_word)
    x_c = jnp.concatenate([x_user, uwa_c[:, N_W:]], axis=1)     # (bu, 256)
    x = x_c / x_c.sum(axis=1, keepdims=True)
    o1 = jnp.einsum("uf,fj->uj", x, W_s1) + b_s1
    o2 = jnp.einsum("uf,fj->uj", sent_c, W_s2) + b_s2
    o1n = jnp.einsum("uj,jk->uk", o1, W_f1) + b_f1
    o2n = jnp.einsum("uj,jk->uk", o2, W_f1) + b_f1
    xt = jnp.concatenate([o1, o2], axis=1)
    xt = jnp.einsum("uf,fj->uj", jax.nn.relu(
        jnp.einsum("uf,fj->uj", xt, W_fa) + b_fa),
        W_fb) + b_fb
    return (o1n, o2n, xt, attn1[None], attn2[None], x)


_COMPILED = None


def _get_compiled():
    global _COMPILED
    if _COMPILED is not None:
        return _COMPILED
    devs = jax.devices()[:NC]
    mesh = Mesh(np.asarray(devs), ("c",))
    rep = P()
    shd = P("c")
    in_specs = (
        rep, rep, rep, rep,      # pern_feature, word_feature, pern_adj, wp_adj
        shd, shd,                # user_word_adj, sentence_embed (batch dim)
        shd, shd, shd, rep,      # w1, a_src1, a_dst1, bias1   (head dim)
        shd, shd, shd, rep,      # w2, a_src2, a_dst2, bias2
        rep, rep, rep, rep, rep, rep, rep, rep, rep, rep,
    )
    out_specs = (shd, shd, shd,  # o1n, o2n, xt   (batch dim)
                 shd, shd,       # attn1, attn2   (head dim)
                 shd)            # x              (batch dim)
    fn = jax.jit(shard_map(_device_fn, mesh=mesh,
                           in_specs=in_specs, out_specs=out_specs,
                           check_rep=False))
    _COMPILED = fn
    return fn


def kernel(**inputs):
    fn = _get_compiled()
    args = (
        inputs["pern_feature"], inputs["word_feature"],
        inputs["pern_adj"], inputs["word_pern_adj"],
        inputs["user_word_adj"], inputs["sentence_embed"],
        inputs["w1"], inputs["a_src1"], inputs["a_dst1"], inputs["bias1"],
        inputs["w2"], inputs["a_src2"], inputs["a_dst2"], inputs["bias2"],
        inputs["W_s1"], inputs["b_s1"], inputs["W_s2"], inputs["b_s2"],
        inputs["W_f1"], inputs["b_f1"], inputs["W_fa"], inputs["b_fa"],
        inputs["W_fb"], inputs["b_fb"],
    )
    args = tuple(np.asarray(a) for a in args)
    global _last_args
    _last_args = args
    out = fn(*args)
    out = jax.tree.map(np.asarray, out)
    o1n, o2n, xt, attn1, attn2, x = out
    return (o1n, o2n, xt, attn1, attn2, x)
